# revision 1
# baseline (speedup 1.0000x reference)
"""Neg-Pearson loss kernel for Trainium2 (raw Bass, no TileContext), 8-core DP.

Problem: preds/labels [B=512, C=4, N=16384] f32 -> scalar
    per-row pearson p over N; per = 1 - sign(p)*p^2 ; output = mean(per).

Sharding: B split across 8 cores (64 B-rows -> 256 (b,c) rows per core).
Each core views its shard as [256, 16384] and streams [128, 4096] tiles.

Engine balance (per-core steady state, measured): DMA floor is ~80 us/pass
(418 GB/s/NC), so both compute engines must stay under it. Chunks alternate
between two op assignments:
  type A (even chunk): DVE: stt(x*y)+accum Sxy, 8x bn_stats(x);  ACT: y^2, y
  type B (odd  chunk): DVE: stt(x*y)+accum Sxy, stt(x*x)+accum Sx2;
                       ACT: y^2, y, copy(x)+accum Sx
This puts DVE at ~78 us and ACT at ~74 us per pass, under the DMA bound.

Final per-row math runs once, entirely on DVE, with
  p^2 = cov^2/(varx*vary), sign(p) = sign(cov):
  per = 1 - (2*[cov>=0]-1) * cov^2 / (varx*vary)
Per-row losses are DMA'd out; host gathers 8x[128,2] and takes the mean.

Raw bass (explicit per-engine programs + semaphores) is used because this
container's walrus build rejects TileContext's exit drain ("Too many sync
wait commands"). The serialized tail works around the DVE write->read
visibility hazard (op N+1 reads stale data written by op N; verified on HW).

`reps` re-streams the whole input R times inside one NEFF (identical result)
so device time can be measured as a slope across R despite ~4 ms of
per-call axon dispatch overhead.
"""

import numpy as np

import concourse.bass as bass
from concourse import mybir
from concourse.bass_utils import run_bass_kernel_spmd

B, C, N = 512, 4, 16384
N_CORES = 8
B_PER_CORE = B // N_CORES                  # 64
ROWS = B_PER_CORE * C                      # 256 rows per core
P = 128                                    # SBUF partitions
N_BLOCKS = ROWS // P                       # 2 row-blocks per core
F = 4096                                   # free-dim chunk (16 KiB/partition)
N_CHUNKS = N // F                          # 4 chunks per row-block
T = N_BLOCKS * N_CHUNKS                    # 8 chunk-iterations per core
BN_F = 512                                 # bn_stats hardware max free size
N_SUB = F // BN_F                          # 8 bn_stats per chunk
NBUF = 4                                   # input slot buffering
NA = N_CHUNKS // 2                         # type-A chunks per block (even i)
NB = N_CHUNKS - NA                         # type-B chunks per block (odd i)
N_FIN_OPS = 27                            # ops in the serialized tail chain

_CACHED_NC = None


def _chunk_src(t):
    """(blk, i, row0, col0) of chunk index t; odd i are type-B chunks."""
    blk, i = divmod(t, N_CHUNKS)
    return blk, i, blk * P, i * F


def build_kernel(reps: int = 1) -> bass.Bass:
    fp32 = mybir.dt.float32
    Alu = mybir.AluOpType
    Act = mybir.ActivationFunctionType

    nc = bass.Bass(name="neg_pearson")
    n_glob = reps * T
    preds = nc.dram_tensor("preds", [ROWS, N], fp32, kind="ExternalInput")
    labels = nc.dram_tensor("labels", [ROWS, N], fp32, kind="ExternalInput")
    out_per = nc.dram_tensor("per", [P, N_BLOCKS], fp32, kind="ExternalOutput")

    with (
        nc.Block() as block,
        nc.semaphore("s_in") as s_in,      # x+y DMA completion (32/chunk)
        nc.semaphore("s_dve") as s_dve,    # DVE chunk completion
        nc.semaphore("s_act") as s_act,    # ACT chunk completion
        nc.semaphore("s_fin") as s_fin,    # serialized tail chain
        nc.sbuf_tensor("xbuf", [P, NBUF, F], fp32) as xbuf,
        nc.sbuf_tensor("ybuf", [P, NBUF, F], fp32) as ybuf,
        nc.sbuf_tensor("dve_junk", [P, F], fp32) as dve_junk,
        nc.sbuf_tensor("act_junk", [P, F], fp32) as act_junk,
        nc.sbuf_tensor("sxy_parts", [P, N_BLOCKS, N_CHUNKS], fp32) as sxy_parts,
        nc.sbuf_tensor("sy_parts", [P, N_BLOCKS, N_CHUNKS], fp32) as sy_parts,
        nc.sbuf_tensor("sy2_parts", [P, N_BLOCKS, N_CHUNKS], fp32) as sy2_parts,
        nc.sbuf_tensor("sxb_parts", [P, N_BLOCKS, NB], fp32) as sxb_parts,
        nc.sbuf_tensor("sx2b_parts", [P, N_BLOCKS, NB], fp32) as sx2b_parts,
        nc.sbuf_tensor("xstats", [P, N_BLOCKS, NA, N_SUB, 6], fp32) as xstats,
        nc.sbuf_tensor("fin", [P, 24, N_BLOCKS], fp32) as fin,
        nc.sbuf_tensor("xmv", [P, N_BLOCKS, 2], fp32) as xmv,
    ):

        @block.sync
        def _(sync):
            for g in range(n_glob):
                blk, i, r0, c0 = _chunk_src(g % T)
                slot = g % NBUF
                if g >= NBUF:
                    # slot reuse: wait for all consumers of chunk g-NBUF
                    sync.wait_ge(s_dve, g - NBUF + 1)
                    sync.wait_ge(s_act, g - NBUF + 1)
                sync.dma_start(
                    ybuf[:, slot, :], labels[r0 : r0 + P, c0 : c0 + F]
                ).then_inc(s_in, 16)
                sync.dma_start(
                    xbuf[:, slot, :], preds[r0 : r0 + P, c0 : c0 + F]
                ).then_inc(s_in, 16)
            # final per-row losses -> DRAM once DVE finished the tail math
            sync.wait_ge(s_fin, N_FIN_OPS)
            sync.dma_start(out_per[:, :], fin[:, 23, :]).then_inc(s_in, 16)
            sync.wait_ge(s_in, 32 * n_glob + 16)

        @block.vector
        def _(vector):
            for g in range(n_glob):
                blk, i, r0, c0 = _chunk_src(g % T)
                slot = g % NBUF
                vector.wait_ge(s_in, 32 * (g + 1))
                vector.scalar_tensor_tensor(
                    out=dve_junk[:, :],
                    in0=xbuf[:, slot, :],
                    scalar=1.0,
                    in1=ybuf[:, slot, :],
                    op0=Alu.bypass,
                    op1=Alu.mult,
                    accum_out=sxy_parts[:, blk, i : i + 1],
                )
                if i % 2 == 0:
                    # type A: bn_stats(x) -> (mean, M2) partials
                    last = None
                    for j in range(N_SUB):
                        last = vector.bn_stats(
                            out=xstats[:, blk, i // 2, j, :],
                            in_=xbuf[:, slot, j * BN_F : (j + 1) * BN_F],
                        )
                    last.then_inc(s_dve, 1)
                else:
                    # type B: Sx2 on DVE (Sx goes to ACT)
                    vector.scalar_tensor_tensor(
                        out=dve_junk[:, :],
                        in0=xbuf[:, slot, :],
                        scalar=1.0,
                        in1=xbuf[:, slot, :],
                        op0=Alu.bypass,
                        op1=Alu.mult,
                        accum_out=sx2b_parts[:, blk, i // 2 : i // 2 + 1],
                    ).then_inc(s_dve, 1)

            # ---- final per-row math, all on DVE ----
            # The DVE write pipe does not interlock with the next op's read:
            # a value written by op N is stale when read by op N+1/N+2
            # (verified on HW). Serialize the tail chain through s_fin.
            vector.wait_ge(s_act, n_glob)
            fin_ops = [0]

            def fgate():
                if fin_ops[0] > 0:
                    vector.wait_ge(s_fin, fin_ops[0])

            def fdone(inst):
                inst.then_inc(s_fin, 1)
                fin_ops[0] += 1

            sxy = fin[:, 0, :]
            sy = fin[:, 1, :]
            sy2 = fin[:, 2, :]
            my = fin[:, 3, :]
            my2 = fin[:, 4, :]
            vary = fin[:, 5, :]
            mxmy = fin[:, 6, :]
            cov = fin[:, 7, :]
            d = fin[:, 8, :]
            rd = fin[:, 9, :]
            c2 = fin[:, 10, :]
            p2 = fin[:, 11, :]
            mask = fin[:, 12, :]
            sgn = fin[:, 13, :]
            tt = fin[:, 14, :]
            sxb = fin[:, 15, :]
            sx2b = fin[:, 16, :]
            q1 = fin[:, 17, :]     # mxA^2
            q2 = fin[:, 18, :]     # varA + mxA^2
            sx = fin[:, 19, :]     # total sum x
            sx2 = fin[:, 20, :]    # total sum x^2
            mx = fin[:, 21, :]
            varx = fin[:, 22, :]
            per = fin[:, 23, :]

            NA_ELEMS = float(NA * F)  # x elements covered by bn_stats per row

            for blk in range(N_BLOCKS):
                fgate()
                fdone(vector.bn_aggr(out=xmv[:, blk, :], in_=xstats[:, blk]))
            mxa = xmv[:, :, 0]
            vara = xmv[:, :, 1]

            fgate()
            fdone(vector.reduce_sum(
                out=sxy, in_=sxy_parts[:, :, :], axis=mybir.AxisListType.X))
            fgate()
            fdone(vector.reduce_sum(
                out=sy, in_=sy_parts[:, :, :], axis=mybir.AxisListType.X))
            fgate()
            fdone(vector.reduce_sum(
                out=sy2, in_=sy2_parts[:, :, :], axis=mybir.AxisListType.X))
            fgate()
            fdone(vector.reduce_sum(
                out=sxb, in_=sxb_parts[:, :, :], axis=mybir.AxisListType.X))
            fgate()
            fdone(vector.reduce_sum(
                out=sx2b, in_=sx2b_parts[:, :, :], axis=mybir.AxisListType.X))

            # x totals: bn_aggr part (NA_ELEMS elems) + type-B accum part
            fgate()
            fdone(vector.scalar_tensor_tensor(
                out=q1, in0=mxa, scalar=1.0, in1=mxa,
                op0=Alu.bypass, op1=Alu.mult))
            fgate()
            fdone(vector.scalar_tensor_tensor(
                out=q2, in0=vara, scalar=1.0, in1=q1,
                op0=Alu.bypass, op1=Alu.add))
            fgate()
            fdone(vector.scalar_tensor_tensor(
                out=sx, in0=mxa, scalar=NA_ELEMS, in1=sxb,
                op0=Alu.mult, op1=Alu.add))
            fgate()
            fdone(vector.scalar_tensor_tensor(
                out=sx2, in0=q2, scalar=NA_ELEMS, in1=sx2b,
                op0=Alu.mult, op1=Alu.add))

            inv_n = 1.0 / N
            fgate()
            fdone(vector.tensor_scalar_mul(out=mx, in0=sx, scalar1=inv_n))
            fgate()
            fdone(vector.tensor_scalar_mul(out=my, in0=sy, scalar1=inv_n))
            fgate()
            fdone(vector.scalar_tensor_tensor(
                out=my2, in0=my, scalar=1.0, in1=my,
                op0=Alu.bypass, op1=Alu.mult))
            fgate()
            fdone(vector.scalar_tensor_tensor(
                out=q1, in0=mx, scalar=1.0, in1=mx,
                op0=Alu.bypass, op1=Alu.mult))
            # varx = sx2/N - mx^2 ; vary = sy2/N - my^2 ; cov = sxy/N - mx*my
            fgate()
            fdone(vector.scalar_tensor_tensor(
                out=varx, in0=sx2, scalar=inv_n, in1=q1,
                op0=Alu.mult, op1=Alu.subtract))
            fgate()
            fdone(vector.scalar_tensor_tensor(
                out=vary, in0=sy2, scalar=inv_n, in1=my2,
                op0=Alu.mult, op1=Alu.subtract))
            fgate()
            fdone(vector.scalar_tensor_tensor(
                out=mxmy, in0=mx, scalar=1.0, in1=my,
                op0=Alu.bypass, op1=Alu.mult))
            fgate()
            fdone(vector.scalar_tensor_tensor(
                out=cov, in0=sxy, scalar=inv_n, in1=mxmy,
                op0=Alu.mult, op1=Alu.subtract))
            # per = 1 - (2*[cov>=0]-1) * cov^2 / (varx*vary)
            fgate()
            fdone(vector.scalar_tensor_tensor(
                out=d, in0=varx, scalar=1.0, in1=vary,
                op0=Alu.bypass, op1=Alu.mult))
            fgate()
            fdone(vector.reciprocal(out=rd, in_=d))
            fgate()
            fdone(vector.scalar_tensor_tensor(
                out=c2, in0=cov, scalar=1.0, in1=cov,
                op0=Alu.bypass, op1=Alu.mult))
            fgate()
            fdone(vector.scalar_tensor_tensor(
                out=p2, in0=c2, scalar=1.0, in1=rd,
                op0=Alu.bypass, op1=Alu.mult))
            fgate()
            fdone(vector.tensor_scalar(
                out=mask, in0=cov, scalar1=0.0, scalar2=None, op0=Alu.is_ge))
            fgate()
            fdone(vector.tensor_scalar(
                out=sgn, in0=mask, scalar1=2.0, scalar2=-1.0,
                op0=Alu.mult, op1=Alu.add))
            fgate()
            fdone(vector.scalar_tensor_tensor(
                out=tt, in0=sgn, scalar=1.0, in1=p2,
                op0=Alu.bypass, op1=Alu.mult))
            fgate()
            fdone(vector.tensor_scalar(
                out=per, in0=tt, scalar1=-1.0, scalar2=1.0,
                op0=Alu.mult, op1=Alu.add))
            assert fin_ops[0] == N_FIN_OPS, fin_ops

        @block.scalar
        def _(scalar):
            for g in range(n_glob):
                blk, i, r0, c0 = _chunk_src(g % T)
                slot = g % NBUF
                scalar.wait_ge(s_in, 32 * (g + 1))
                scalar.activation(
                    out=act_junk[:, :],
                    in_=ybuf[:, slot, :],
                    func=Act.Square,
                    accum_out=sy2_parts[:, blk, i : i + 1],
                )
                inst = scalar.activation(
                    out=act_junk[:, :],
                    in_=ybuf[:, slot, :],
                    func=Act.Copy,
                    accum_out=sy_parts[:, blk, i : i + 1],
                )
                if i % 2 == 1:
                    # type B: Sx on ACT
                    inst = scalar.activation(
                        out=act_junk[:, :],
                        in_=xbuf[:, slot, :],
                        func=Act.Copy,
                        accum_out=sxb_parts[:, blk, i // 2 : i // 2 + 1],
                    )
                inst.then_inc(s_act, 1)

    return nc


def _get_nc() -> bass.Bass:
    global _CACHED_NC
    if _CACHED_NC is None:
        _CACHED_NC = build_kernel()
    return _CACHED_NC


def shard_inputs(preds: np.ndarray, labels: np.ndarray) -> list[dict[str, np.ndarray]]:
    preds = np.asarray(preds, dtype=np.float32).reshape(B, C, N)
    labels = np.asarray(labels, dtype=np.float32).reshape(B, C, N)
    in_maps = []
    for c in range(N_CORES):
        sl = slice(c * B_PER_CORE, (c + 1) * B_PER_CORE)
        in_maps.append(
            {
                "preds": np.ascontiguousarray(preds[sl].reshape(ROWS, N)),
                "labels": np.ascontiguousarray(labels[sl].reshape(ROWS, N)),
            }
        )
    return in_maps


def run(preds: np.ndarray, labels: np.ndarray, **run_kwargs):
    """Run the SPMD kernel; returns (scalar ndarray, BassKernelResults)."""
    nc = _get_nc()
    res = run_bass_kernel_spmd(
        nc, shard_inputs(preds, labels), core_ids=list(range(N_CORES)), **run_kwargs
    )
    vals = np.concatenate([r["per"].reshape(-1) for r in res.results])
    out = np.asarray(vals.astype(np.float64).mean(), dtype=np.float32)
    return out, res


def kernel(preds: np.ndarray, labels: np.ndarray) -> np.ndarray:
    out, _ = run(preds, labels)
    return out



# revision 2
# speedup vs baseline: 1.5508x; 1.5508x over previous
"""Neg-Pearson loss kernel for Trainium2 (raw Bass, no TileContext), 8-core DP.

Problem: preds/labels [B=512, C=4, N=16384] f32 -> scalar
    per-row pearson p over N; per = 1 - sign(p)*p^2 ; output = mean(per).

Sharding: B split across 8 cores (64 B-rows -> 256 (b,c) rows per core).
Each core views its shard as [256, 16384] and streams [128, 4096] tiles
with NBUF=5 input slots per tensor.

Engine balance (per-core steady state; per-pass times HW-measured via
engine-only microbenches): the kernel is DMA-bound — the streaming floor
is 33.55 MB / pass at whatever HBM rate the (shared) host currently
gives (~74 us calm, ~96+ us contended). Compute is kept safely under
that floor on BOTH engines:
  DVE (~67 us/pass): stt(x*y)+accum Sxy every chunk; bn_stats covers
      x-stats (Sx, Sx2 via mean/M2) for all of x except the last 2048
      columns of each row-block's final chunk.
  ACT (~68 us/pass): Square(y)+accum Sy2, Copy(y)+accum Sy every chunk;
      Square(x)/Copy(x) on the 2048-column x-tail of each block.
An earlier split (bn_stats on even chunks only, x^2/copy-x on odd) put
ACT at ~76 us/pass, which became the critical path whenever the host's
HBM was uncontended; a bn_stats chunk costs the same as an stt chunk on
DVE (~4.5 us), so moving nearly all x-stats to bn_stats shaved ACT by
~8 us at no DVE cost.

Final per-row math runs once, entirely on DVE, with
  p^2 = cov^2/(varx*vary), sign(p) = sign(cov):
  per = 1 - (2*[cov>=0]-1) * cov^2 / (varx*vary)
Per-row losses are DMA'd out; host gathers 8x[128,2] and takes the mean.

Raw bass (explicit per-engine programs + semaphores) is used because this
container's walrus build rejects TileContext's exit drain ("Too many sync
wait commands"). The serialized tail works around the DVE write->read
visibility hazard (op N+1 reads stale data written by op N; verified on HW).

`reps` re-streams the whole input R times inside one NEFF (identical result)
so device time can be measured as a slope across R despite several ms of
per-call axon dispatch overhead.
"""

import numpy as np

import concourse.bass as bass
from concourse import mybir
from concourse.bass_utils import run_bass_kernel_spmd

B, C, N = 512, 4, 16384
N_CORES = 8
B_PER_CORE = B // N_CORES                  # 64
ROWS = B_PER_CORE * C                      # 256 rows per core
P = 128                                    # SBUF partitions
N_BLOCKS = ROWS // P                       # 2 row-blocks per core
F = 4096                                   # free-dim chunk (16 KiB/partition)
N_CHUNKS = N // F                          # 4 chunks per row-block
T = N_BLOCKS * N_CHUNKS                    # 8 chunk-iterations per core
BN_F = 512                                 # bn_stats hardware max free size
NBUF = 5                                   # input slot buffering
ACT_X_ELEMS = 2048                         # x-tail columns handled by ACT
N_SEG_FULL = F // BN_F                     # bn segments per full chunk
N_SEG_LAST = (F - ACT_X_ELEMS) // BN_F     # bn segments in a block's last chunk
N_SEG_BLK = (N_CHUNKS - 1) * N_SEG_FULL + N_SEG_LAST
BN_ELEMS = float((N_CHUNKS - 1) * F + (F - ACT_X_ELEMS))  # x elems per row via bn
N_FIN_OPS = 25                             # ops in the serialized tail chain

_CACHED_NC = None


def _chunk_src(t):
    """(blk, i, row0, col0) of chunk index t."""
    blk, i = divmod(t, N_CHUNKS)
    return blk, i, blk * P, i * F


def build_kernel(reps: int = 1) -> bass.Bass:
    fp32 = mybir.dt.float32
    Alu = mybir.AluOpType
    Act = mybir.ActivationFunctionType

    nc = bass.Bass(name="neg_pearson")
    n_glob = reps * T
    preds = nc.dram_tensor("preds", [ROWS, N], fp32, kind="ExternalInput")
    labels = nc.dram_tensor("labels", [ROWS, N], fp32, kind="ExternalInput")
    out_per = nc.dram_tensor("per", [P, N_BLOCKS], fp32, kind="ExternalOutput")

    with (
        nc.Block() as block,
        nc.semaphore("s_in") as s_in,      # x+y DMA completion (32/chunk)
        nc.semaphore("s_dve") as s_dve,    # DVE chunk completion
        nc.semaphore("s_act") as s_act,    # ACT chunk completion
        nc.semaphore("s_fin") as s_fin,    # serialized tail chain
        nc.sbuf_tensor("xbuf", [P, NBUF, F], fp32) as xbuf,
        nc.sbuf_tensor("ybuf", [P, NBUF, F], fp32) as ybuf,
        nc.sbuf_tensor("dve_junk", [P, F], fp32) as dve_junk,
        nc.sbuf_tensor("act_junk", [P, F], fp32) as act_junk,
        nc.sbuf_tensor("sxy_parts", [P, N_BLOCKS, N_CHUNKS], fp32) as sxy_parts,
        nc.sbuf_tensor("sy_parts", [P, N_BLOCKS, N_CHUNKS], fp32) as sy_parts,
        nc.sbuf_tensor("sy2_parts", [P, N_BLOCKS, N_CHUNKS], fp32) as sy2_parts,
        nc.sbuf_tensor("sxc_parts", [P, N_BLOCKS, 1], fp32) as sxc_parts,
        nc.sbuf_tensor("sx2c_parts", [P, N_BLOCKS, 1], fp32) as sx2c_parts,
        nc.sbuf_tensor("xstats", [P, N_BLOCKS, N_SEG_BLK, 6], fp32) as xstats,
        nc.sbuf_tensor("fin", [P, 24, N_BLOCKS], fp32) as fin,
        nc.sbuf_tensor("xmv", [P, N_BLOCKS, 2], fp32) as xmv,
    ):

        @block.sync
        def _(sync):
            for g in range(n_glob):
                blk, i, r0, c0 = _chunk_src(g % T)
                slot = g % NBUF
                if g >= NBUF:
                    # slot reuse: wait for all consumers of chunk g-NBUF
                    sync.wait_ge(s_dve, g - NBUF + 1)
                    sync.wait_ge(s_act, g - NBUF + 1)
                sync.dma_start(
                    ybuf[:, slot, :], labels[r0 : r0 + P, c0 : c0 + F]
                ).then_inc(s_in, 16)
                sync.dma_start(
                    xbuf[:, slot, :], preds[r0 : r0 + P, c0 : c0 + F]
                ).then_inc(s_in, 16)
            # final per-row losses -> DRAM once DVE finished the tail math
            sync.wait_ge(s_fin, N_FIN_OPS)
            sync.dma_start(out_per[:, :], fin[:, 23, :]).then_inc(s_in, 16)
            sync.wait_ge(s_in, 32 * n_glob + 16)

        @block.vector
        def _(vector):
            for g in range(n_glob):
                blk, i, r0, c0 = _chunk_src(g % T)
                slot = g % NBUF
                is_last = i == N_CHUNKS - 1
                n_seg = N_SEG_LAST if is_last else N_SEG_FULL
                vector.wait_ge(s_in, 32 * (g + 1))
                inst = vector.scalar_tensor_tensor(
                    out=dve_junk[:, :],
                    in0=xbuf[:, slot, :],
                    scalar=1.0,
                    in1=ybuf[:, slot, :],
                    op0=Alu.bypass,
                    op1=Alu.mult,
                    accum_out=sxy_parts[:, blk, i : i + 1],
                )
                seg0 = i * N_SEG_FULL
                for j in range(n_seg):
                    inst = vector.bn_stats(
                        out=xstats[:, blk, seg0 + j, :],
                        in_=xbuf[:, slot, j * BN_F : (j + 1) * BN_F],
                    )
                inst.then_inc(s_dve, 1)

            # ---- final per-row math, all on DVE ----
            # The DVE write pipe does not interlock with the next op's read:
            # a value written by op N is stale when read by op N+1/N+2
            # (verified on HW). Serialize the tail chain through s_fin.
            vector.wait_ge(s_act, n_glob)
            fin_ops = [0]

            def fgate():
                if fin_ops[0] > 0:
                    vector.wait_ge(s_fin, fin_ops[0])

            def fdone(inst):
                inst.then_inc(s_fin, 1)
                fin_ops[0] += 1

            sxy = fin[:, 0, :]
            sy = fin[:, 1, :]
            sy2 = fin[:, 2, :]
            my = fin[:, 3, :]
            my2 = fin[:, 4, :]
            vary = fin[:, 5, :]
            mxmy = fin[:, 6, :]
            cov = fin[:, 7, :]
            d = fin[:, 8, :]
            rd = fin[:, 9, :]
            c2 = fin[:, 10, :]
            p2 = fin[:, 11, :]
            mask = fin[:, 12, :]
            sgn = fin[:, 13, :]
            tt = fin[:, 14, :]
            q1 = fin[:, 17, :]
            q2 = fin[:, 18, :]
            sx = fin[:, 19, :]
            sx2 = fin[:, 20, :]
            mx = fin[:, 21, :]
            varx = fin[:, 22, :]
            per = fin[:, 23, :]

            for blk in range(N_BLOCKS):
                fgate()
                fdone(vector.bn_aggr(out=xmv[:, blk, :], in_=xstats[:, blk]))
            mxa = xmv[:, :, 0]
            vara = xmv[:, :, 1]

            fgate()
            fdone(vector.reduce_sum(
                out=sxy, in_=sxy_parts[:, :, :], axis=mybir.AxisListType.X))
            fgate()
            fdone(vector.reduce_sum(
                out=sy, in_=sy_parts[:, :, :], axis=mybir.AxisListType.X))
            fgate()
            fdone(vector.reduce_sum(
                out=sy2, in_=sy2_parts[:, :, :], axis=mybir.AxisListType.X))

            # x totals: bn_aggr part (BN_ELEMS elems/row) + ACT-covered tail
            fgate()
            fdone(vector.scalar_tensor_tensor(
                out=q1, in0=mxa, scalar=1.0, in1=mxa,
                op0=Alu.bypass, op1=Alu.mult))
            fgate()
            fdone(vector.scalar_tensor_tensor(
                out=q2, in0=vara, scalar=1.0, in1=q1,
                op0=Alu.bypass, op1=Alu.add))
            fgate()
            fdone(vector.scalar_tensor_tensor(
                out=sx, in0=mxa, scalar=BN_ELEMS, in1=sxc_parts[:, :, 0],
                op0=Alu.mult, op1=Alu.add))
            fgate()
            fdone(vector.scalar_tensor_tensor(
                out=sx2, in0=q2, scalar=BN_ELEMS, in1=sx2c_parts[:, :, 0],
                op0=Alu.mult, op1=Alu.add))

            inv_n = 1.0 / N
            fgate()
            fdone(vector.tensor_scalar_mul(out=mx, in0=sx, scalar1=inv_n))
            fgate()
            fdone(vector.tensor_scalar_mul(out=my, in0=sy, scalar1=inv_n))
            fgate()
            fdone(vector.scalar_tensor_tensor(
                out=my2, in0=my, scalar=1.0, in1=my,
                op0=Alu.bypass, op1=Alu.mult))
            fgate()
            fdone(vector.scalar_tensor_tensor(
                out=q1, in0=mx, scalar=1.0, in1=mx,
                op0=Alu.bypass, op1=Alu.mult))
            # varx = sx2/N - mx^2 ; vary = sy2/N - my^2 ; cov = sxy/N - mx*my
            fgate()
            fdone(vector.scalar_tensor_tensor(
                out=varx, in0=sx2, scalar=inv_n, in1=q1,
                op0=Alu.mult, op1=Alu.subtract))
            fgate()
            fdone(vector.scalar_tensor_tensor(
                out=vary, in0=sy2, scalar=inv_n, in1=my2,
                op0=Alu.mult, op1=Alu.subtract))
            fgate()
            fdone(vector.scalar_tensor_tensor(
                out=mxmy, in0=mx, scalar=1.0, in1=my,
                op0=Alu.bypass, op1=Alu.mult))
            fgate()
            fdone(vector.scalar_tensor_tensor(
                out=cov, in0=sxy, scalar=inv_n, in1=mxmy,
                op0=Alu.mult, op1=Alu.subtract))
            # per = 1 - (2*[cov>=0]-1) * cov^2 / (varx*vary)
            fgate()
            fdone(vector.scalar_tensor_tensor(
                out=d, in0=varx, scalar=1.0, in1=vary,
                op0=Alu.bypass, op1=Alu.mult))
            fgate()
            fdone(vector.reciprocal(out=rd, in_=d))
            fgate()
            fdone(vector.scalar_tensor_tensor(
                out=c2, in0=cov, scalar=1.0, in1=cov,
                op0=Alu.bypass, op1=Alu.mult))
            fgate()
            fdone(vector.scalar_tensor_tensor(
                out=p2, in0=c2, scalar=1.0, in1=rd,
                op0=Alu.bypass, op1=Alu.mult))
            fgate()
            fdone(vector.tensor_scalar(
                out=mask, in0=cov, scalar1=0.0, scalar2=None, op0=Alu.is_ge))
            fgate()
            fdone(vector.tensor_scalar(
                out=sgn, in0=mask, scalar1=2.0, scalar2=-1.0,
                op0=Alu.mult, op1=Alu.add))
            fgate()
            fdone(vector.scalar_tensor_tensor(
                out=tt, in0=sgn, scalar=1.0, in1=p2,
                op0=Alu.bypass, op1=Alu.mult))
            fgate()
            fdone(vector.tensor_scalar(
                out=per, in0=tt, scalar1=-1.0, scalar2=1.0,
                op0=Alu.mult, op1=Alu.add))
            assert fin_ops[0] == N_FIN_OPS, fin_ops

        @block.scalar
        def _(scalar):
            for g in range(n_glob):
                blk, i, r0, c0 = _chunk_src(g % T)
                slot = g % NBUF
                is_last = i == N_CHUNKS - 1
                scalar.wait_ge(s_in, 32 * (g + 1))
                inst = scalar.activation(
                    out=act_junk[:, :],
                    in_=ybuf[:, slot, :],
                    func=Act.Square,
                    accum_out=sy2_parts[:, blk, i : i + 1],
                )
                inst = scalar.activation(
                    out=act_junk[:, :],
                    in_=ybuf[:, slot, :],
                    func=Act.Copy,
                    accum_out=sy_parts[:, blk, i : i + 1],
                )
                if is_last:
                    # x-stats for the block's x-tail (last ACT_X_ELEMS cols)
                    x_tail = xbuf[:, slot, F - ACT_X_ELEMS : F]
                    inst = scalar.activation(
                        out=act_junk[:, 0:ACT_X_ELEMS],
                        in_=x_tail,
                        func=Act.Square,
                        accum_out=sx2c_parts[:, blk, 0:1],
                    )
                    inst = scalar.activation(
                        out=act_junk[:, 0:ACT_X_ELEMS],
                        in_=x_tail,
                        func=Act.Copy,
                        accum_out=sxc_parts[:, blk, 0:1],
                    )
                inst.then_inc(s_act, 1)

    return nc


def _get_nc() -> bass.Bass:
    global _CACHED_NC
    if _CACHED_NC is None:
        _CACHED_NC = build_kernel()
    return _CACHED_NC


def shard_inputs(preds: np.ndarray, labels: np.ndarray) -> list[dict[str, np.ndarray]]:
    preds = np.asarray(preds, dtype=np.float32).reshape(B, C, N)
    labels = np.asarray(labels, dtype=np.float32).reshape(B, C, N)
    in_maps = []
    for c in range(N_CORES):
        sl = slice(c * B_PER_CORE, (c + 1) * B_PER_CORE)
        in_maps.append(
            {
                "preds": np.ascontiguousarray(preds[sl].reshape(ROWS, N)),
                "labels": np.ascontiguousarray(labels[sl].reshape(ROWS, N)),
            }
        )
    return in_maps


def run(preds: np.ndarray, labels: np.ndarray, **run_kwargs):
    """Run the SPMD kernel; returns (scalar ndarray, BassKernelResults)."""
    nc = _get_nc()
    res = run_bass_kernel_spmd(
        nc, shard_inputs(preds, labels), core_ids=list(range(N_CORES)), **run_kwargs
    )
    vals = np.concatenate([r["per"].reshape(-1) for r in res.results])
    out = np.asarray(vals.astype(np.float64).mean(), dtype=np.float32)
    return out, res


def kernel(preds: np.ndarray, labels: np.ndarray) -> np.ndarray:
    out, _ = run(preds, labels)
    return out
